# revision 8
# baseline (speedup 1.0000x reference)
"""Trainium2 Bass kernel for nn_BehaviorEngine (gnn_message_passing).

Per-cell computation over a 1024x1024 grid, D=32:
  cube = A^3 (elementwise); self_pattern = cube/(||cube||+eps)*||A||
  cwp  = A + 0.1*self_pattern
  x    = [cwp ; potential]  (33)
  h    = relu(x @ W1 + b1)  (64)
  out  = h @ W2 + b2 + 0.3*A

Sharding: pure data parallel over H across 8 cores (131072 cells/core).

Per-core dataflow (all fp32):
  - natural layout [128 cells-on-partition, j, 32] blocks of 2048 cells
  - norm math on DVE/ACT in natural layout (grouped reductions via 3D APs)
  - x33 = [cwp | pot] tiles PE-transposed into PSUM at partitions 64:97
  - one merged matmul per 512-cell tile with block lhsT [97,96]:
      out[0:64]  = W1ext.T @ xT_i        (h_pre of tile i)
      out[64:96] = W2.T    @ hT_{i-1}    (transformed.T of tile i-1)
  - relu+b1 evac (ACT) feeds next tile's rhs rows 0:64
  - outT+b2 evac (DVE) -> stage, PE-transposed back to natural [128,32]
  - final = 0.3*A + psum (fused scalar_tensor_tensor), DMA out
"""

import sys

sys.path.insert(0, "/opt/trn_rl_repo")

from contextlib import ExitStack

import numpy as np

import concourse.bass as bass
import concourse.tile as tile
from concourse import bacc, mybir
from concourse._compat import with_exitstack
from concourse.bass_utils import run_bass_kernel_spmd

F32 = mybir.dt.float32
AF = mybir.ActivationFunctionType
OP = mybir.AluOpType

H, W, D, HID = 1024, 1024, 32, 64
NCORES = 8
P = 128
JPC = (H // NCORES) * W // P  # 1024 j-positions per partition per core
JPB = 16                      # j's per block
BLK = P * JPB                 # 2048 cells per block
NBLK = JPC // JPB             # 64 blocks
TPB = 4                       # 512-cell mm tiles per block
NT = NBLK * TPB               # 256 mm tiles
TN = 512                      # cells per mm tile
EPS = 1e-8
ALPHA = 0.3
PATTERN_W = 0.1


@with_exitstack
def _body(ctx: ExitStack, tc: tile.TileContext, gv, ppv, w1v, b1v, w2v, b2v, ov, identv, nblk=NBLK):
    nc = tc.nc

    const = ctx.enter_context(tc.tile_pool(name="const", bufs=1))
    apool = ctx.enter_context(tc.tile_pool(name="a", bufs=3))
    epool = ctx.enter_context(tc.tile_pool(name="elw", bufs=2))
    spool = ctx.enter_context(tc.tile_pool(name="small", bufs=2))
    x33p = ctx.enter_context(tc.tile_pool(name="x33", bufs=2))
    rhsp = ctx.enter_context(tc.tile_pool(name="rhs", bufs=1))
    stgp = ctx.enter_context(tc.tile_pool(name="stg", bufs=2))
    outp = ctx.enter_context(tc.tile_pool(name="osb", bufs=2))
    ps_x = ctx.enter_context(tc.tile_pool(name="psx", bufs=2, space="PSUM"))
    ps_m = ctx.enter_context(tc.tile_pool(name="psm", bufs=2, space="PSUM"))
    ps_n = ctx.enter_context(tc.tile_pool(name="psn", bufs=2, space="PSUM"))

    # --- constants ---
    idt = const.tile([P, 160], F32)  # [:,0:128] = I128; [64:96,128:160] = I32
    nc.sync.dma_start(idt[:, 0:128], identv[:, :])
    nc.sync.dma_start(idt[0:32, 128:160], identv[0:32, 0:32])

    wblk = const.tile([P, 128], F32)  # [0:33,64:128]=W1 ; [64:128,0:32]=W2
    nc.vector.memset(wblk[:], 0.0)
    nc.sync.dma_start(wblk[64:128, 0:32], w2v[:, :])
    nc.sync.dma_start(wblk[0:33, 64:128], w1v[:, :])

    bcol = const.tile([P, 1], F32)  # rows 0:32 = b2; rows 64:128 = b1
    nc.vector.memset(bcol[:], 0.0)
    nc.sync.dma_start(bcol[64:128, 0:1], b1v.rearrange("(h one) -> h one", one=1))
    nc.sync.dma_start(bcol[0:32, 0:1], b2v.rearrange("(d one) -> d one", one=1))

    rhs_slots = []
    for k in range(3):
        rslot = rhsp.tile([P, TN], F32, tag=f"rhs{k}", name=f"rhs{k}")
        rhs_slots.append(rslot)
    for r in rhs_slots:
        nc.vector.memset(r[32:64, :], 0.0)
        nc.vector.memset(r[64:128, :], 0.0)

    prev_mm = None   # psum of previous tile's matmul
    blk_state = {}   # block -> (A tile, psum_nat tile)

    def input_block(b):
        a = apool.tile([P, JPB * D], F32, tag="a")
        a3 = a[:].rearrange("p (j d) -> p j d", d=D)
        nc.sync.dma_start(a3, gv[:, JPB * b : JPB * (b + 1), :])
        pp = spool.tile([P, JPB], F32, tag="pp")
        nc.sync.dma_start(pp[:], ppv[:, JPB * b : JPB * (b + 1)])

        sq = epool.tile([P, JPB * D], F32, tag="sq")
        nc.scalar.activation(sq[:], a[:], AF.Square)
        cube = epool.tile([P, JPB * D], F32, tag="cube")
        nc.vector.tensor_mul(cube[:], sq[:], a[:])
        six = epool.tile([P, JPB * D], F32, tag="six")
        nc.scalar.activation(six[:], cube[:], AF.Square)

        s2 = spool.tile([P, JPB], F32, tag="s2")
        nc.vector.tensor_reduce(
            s2[:], sq[:].rearrange("p (j d) -> p j d", d=D),
            axis=mybir.AxisListType.X, op=OP.add)
        s6 = spool.tile([P, JPB], F32, tag="s6")
        nc.vector.tensor_reduce(
            s6[:], six[:].rearrange("p (j d) -> p j d", d=D),
            axis=mybir.AxisListType.X, op=OP.add)

        cn = spool.tile([P, JPB], F32, tag="cn")
        nc.scalar.activation(cn[:], s2[:], AF.Sqrt)
        c6 = spool.tile([P, JPB], F32, tag="c6")
        nc.scalar.activation(c6[:], s6[:], AF.Sqrt)
        nc.vector.tensor_scalar_add(c6[:], c6[:], EPS)
        inv = spool.tile([P, JPB], F32, tag="inv")
        nc.vector.reciprocal(inv[:], c6[:])
        rb = spool.tile([P, JPB], F32, tag="rb")
        nc.vector.scalar_tensor_tensor(
            rb[:], cn[:], PATTERN_W, inv[:], op0=OP.mult, op1=OP.mult)

        # x33[:, j, 0:32] = A + rb*cube ; x33[:, j, 32] = pot
        x33 = x33p.tile([P, JPB * 33], F32, tag="x33")
        x3 = x33[:].rearrange("p (j e) -> p j e", e=33)
        rbb = rb[:].rearrange("p (j one) -> p j one", one=1).broadcast_to((P, JPB, D))
        tmp = epool.tile([P, JPB * D], F32, tag="tmp")
        nc.vector.tensor_mul(tmp[:].rearrange("p (j d) -> p j d", d=D),
                             cube[:].rearrange("p (j d) -> p j d", d=D), rbb)
        nc.vector.tensor_add(
            x3[:, :, 0:D],
            a3,
            tmp[:].rearrange("p (j d) -> p j d", d=D))
        nc.vector.tensor_copy(
            x3[:, :, D : D + 1],
            pp[:].rearrange("p (j one) -> p j one", one=1))

        pn = ps_n.tile([P, JPB * D], F32, tag="psn")
        blk_state[b] = (a, pn)
        return x33

    nt = nblk * TPB
    x33_cur = None
    for i in range(nt + 1):
        b, t = divmod(i, TPB)
        if i < nt:
            if t == 0:
                x33_cur = input_block(b)
            x3 = x33_cur[:].rearrange("p (j e) -> p j e", e=33)
            # forward transposes: 4 chunks of 128 cells -> psum rows 64:97
            px = ps_x.tile([P, TN], F32, tag="psx")
            for c in range(4):
                j = 4 * t + c
                nc.tensor.transpose(
                    px[0:33, 128 * c : 128 * (c + 1)],
                    x3[:, j, :],
                    idt[:, 0:128],
                    tile_position=(0, 0),
                )
            rhs = rhs_slots[i % 3]
            nc.vector.tensor_copy(rhs[0:33, :], px[0:33, :])
            if prev_mm is not None:
                nc.scalar.activation(
                    rhs[64:128, :], prev_mm[64:128, :], AF.Relu, bias=bcol[64:128, 0:1])
        else:
            rhs = rhs_slots[i % 3]  # tail: xT rows stale
            nc.scalar.activation(
                rhs[64:128, :], prev_mm[64:128, :], AF.Relu, bias=bcol[64:128, 0:1])

        pm = ps_m.tile([P, TN], F32, tag="psm")
        nc.tensor.matmul(pm[0:128, :], lhsT=wblk[0:128, 0:128], rhs=rhs[0:128, :],
                         start=True, stop=True)

        if i > 0:
            # output side for tile i-1 (its outT is in pm[64:96])
            bb, tt = divmod(i - 1, TPB)
            a_blk, pn = blk_state[bb]
            stg = stgp.tile([P, TN], F32, tag="stg")
            nc.vector.tensor_scalar_add(stg[0:32, :], pm[0:32, :], bcol[0:32, 0:1])
            for c in range(4):
                j = 4 * tt + c
                nc.tensor.transpose(
                    pn[:, D * j : D * (j + 1)],
                    stg[0:32, 128 * c : 128 * (c + 1)],
                    idt[0:32, 128:160],
                    tile_position=(0, 0),
                )
            if tt == TPB - 1:
                osb = outp.tile([P, JPB * D], F32, tag="osb")
                nc.vector.scalar_tensor_tensor(
                    osb[:], a_blk[:], ALPHA, pn[:], op0=OP.mult, op1=OP.add)
                nc.sync.dma_start(
                    ov[:, JPB * bb : JPB * (bb + 1), :],
                    osb[:].rearrange("p (j d) -> p j d", d=D))
                del blk_state[bb]

        prev_mm = pm


_CACHE = {}


def _build(nblk=NBLK):
    if nblk in _CACHE:
        return _CACHE[nblk]
    nc = bacc.Bacc("TRN2", target_bir_lowering=False, debug=False,
                   enable_asserts=False, num_devices=NCORES)
    jpc = nblk * JPB
    g = nc.dram_tensor("g", [P, jpc, D], F32, kind="ExternalInput")
    pp = nc.dram_tensor("pp", [P, jpc], F32, kind="ExternalInput")
    w1 = nc.dram_tensor("w1", [D + 1, HID], F32, kind="ExternalInput")
    b1 = nc.dram_tensor("b1", [HID], F32, kind="ExternalInput")
    w2 = nc.dram_tensor("w2", [HID, D], F32, kind="ExternalInput")
    b2 = nc.dram_tensor("b2", [D], F32, kind="ExternalInput")
    out = nc.dram_tensor("out", [P, jpc, D], F32, kind="ExternalOutput")
    ident = nc.inline_tensor(np.eye(P, dtype=np.float32), name="ident")
    with tile.TileContext(nc) as tc:
        _body(tc, g.ap(), pp.ap(), w1.ap(), b1.ap(), w2.ap(), b2.ap(),
              out.ap(), ident.ap(), nblk=nblk)
    nc.compile()
    _CACHE[nblk] = nc
    return nc


def kernel(grid_states, potentials, W1, b1, W2, b2):
    nc = _build()
    g = np.asarray(grid_states, dtype=np.float32)
    p = np.asarray(potentials, dtype=np.float32)
    rows = H // NCORES
    in_maps = []
    for c in range(NCORES):
        in_maps.append({
            "g": np.ascontiguousarray(
                g[c * rows : (c + 1) * rows].reshape(P, JPC, D)),
            "pp": np.ascontiguousarray(
                p[c * rows : (c + 1) * rows].reshape(P, JPC)),
            "w1": np.asarray(W1, dtype=np.float32),
            "b1": np.asarray(b1, dtype=np.float32),
            "w2": np.asarray(W2, dtype=np.float32),
            "b2": np.asarray(b2, dtype=np.float32),
        })
    import os
    trace = bool(int(os.environ.get("BENG_TRACE", "0")))
    res = run_bass_kernel_spmd(nc, in_maps, core_ids=list(range(NCORES)),
                               trace=trace)
    _CACHE["last_res"] = res
    outs = [res.results[c]["out"].reshape(rows, W, D) for c in range(NCORES)]
    return np.concatenate(outs, axis=0)


if __name__ == "__main__":
    rng = np.random.default_rng(0)
    gs = rng.standard_normal((H, W, D), dtype=np.float32)
    po = rng.random((H, W), dtype=np.float32)
    W1a = rng.standard_normal((D + 1, HID), dtype=np.float32) * 0.1
    b1a = rng.standard_normal((HID,), dtype=np.float32) * 0.1
    W2a = rng.standard_normal((HID, D), dtype=np.float32) * 0.1
    b2a = rng.standard_normal((D,), dtype=np.float32) * 0.1
    o = kernel(gs, po, W1a, b1a, W2a, b2a)
    print(o.shape, o.dtype)
